# revision 34
# baseline (speedup 1.0000x reference)
"""Multi-head attention (B=2, S=2048, D=1024, H=16) on 8 Trainium2 cores.

Sharding: core c = (batch b, head-group hg) with b = c // 4, hg = c % 4.
Each core computes 4 heads of one batch element end-to-end:
  - Q^T/K^T projections in [dh, s] layout (scores computed transposed so the
    softmax denominator comes out of the PV matmul via a ones-column in V)
  - V projection in natural [s, dh] layout
  - exp on ScalarE with the 1/sqrt(dh) scale fused into the activation
  - partial output projection against the core's row-slice of Wo
Host sums the 4 partial projections per batch and adds bo.

Matmuls run as float32r (full-rate fp32 path on the PE for moving dim >= 256);
accumulation is always fp32 in PSUM. Walrus requires fp32r matmul operands to
be produced by an instruction that rounds to fp32r, so every matmul input tile
is allocated with dtype float32r and written by a DVE/ACT op (the rounding is
fused into copies we need anyway). Input transposes run in plain fp32.
"""

import numpy as np

import concourse.bacc as bacc
import concourse.mybir as mybir
import concourse.tile as tile
from concourse.bass_utils import run_bass_kernel_spmd
from concourse.masks import make_identity

F32 = mybir.dt.float32
F32R = mybir.dt.float32r

S_FULL, D_FULL, NH_PER_CORE, DH = 2048, 1024, 4, 64
N_CORES = 8
B_FULL, H_FULL = 2, 16


def build_core_program(S=S_FULL, D=D_FULL, NH=NH_PER_CORE):
    """One core's program: inputs xq/xk/xv [S,D], weight slices wq/wk/wv
    [D,NSL], wo [NSL,D], biases [NSL]; output out [S,D] (partial sum)."""
    NSL = NH * DH            # projection slice width for this core
    P = 128
    KD = D // P              # d-tiles (contraction tiles for projections)
    NT = NSL // P            # n-tiles = head-pairs
    ST = S // P              # s-tiles
    SBLK = 512 if S % 512 == 0 else S
    NB = S // SBLK           # s/i blocks
    JT = ST                  # j-tiles in attention
    JC = 2                   # j-tiles per score/exp chunk
    SS = SBLK // P           # s-subtiles per block

    nc = bacc.Bacc("TRN2", target_bir_lowering=False, debug=False)

    xq_d = nc.dram_tensor("xq", [S, D], F32, kind="ExternalInput")
    xk_d = nc.dram_tensor("xk", [S, D], F32, kind="ExternalInput")
    xv_d = nc.dram_tensor("xv", [S, D], F32, kind="ExternalInput")
    wq_d = nc.dram_tensor("wq", [D, NSL], F32, kind="ExternalInput")
    wk_d = nc.dram_tensor("wk", [D, NSL], F32, kind="ExternalInput")
    wv_d = nc.dram_tensor("wv", [D, NSL], F32, kind="ExternalInput")
    wo_d = nc.dram_tensor("wo", [NSL, D], F32, kind="ExternalInput")
    bq_d = nc.dram_tensor("bq", [NSL], F32, kind="ExternalInput")
    bk_d = nc.dram_tensor("bk", [NSL], F32, kind="ExternalInput")
    bv_d = nc.dram_tensor("bv", [NSL], F32, kind="ExternalInput")
    out_d = nc.dram_tensor("out", [S, D], F32, kind="ExternalOutput")

    with tile.TileContext(nc) as tc:
        with tc.tile_pool(name="persist", bufs=1) as pp:
            ident = pp.tile([P, P], F32)
            make_identity(nc, ident)


            # Weights: DMA fp32 staging -> rounded fp32r copies.
            wq_sb = pp.tile([P, KD, NSL], F32R)
            wk_sb = pp.tile([P, KD, NSL], F32R)
            wv_sb = pp.tile([P, KD, NSL], F32R)
            wo_sb = pp.tile([P, NT, D], F32R)
            bq_sb = pp.tile([P, NT], F32)
            nc.sync.dma_start(bq_sb, bq_d.rearrange("(t p) -> p t", p=P))
            bk_sb = pp.tile([P, NT], F32)
            nc.sync.dma_start(bk_sb, bk_d.rearrange("(t p) -> p t", p=P))
            bv_sb = pp.tile([P, NT], F32)
            nc.sync.dma_start(bv_sb, bv_d.rearrange("(t p) -> p t", p=P))

            with tc.tile_pool(name="wstage", bufs=2) as wsp:
                for w_d, w_sb, wkd, wn in (
                    (wq_d, wq_sb, KD, NSL),
                    (wk_d, wk_sb, KD, NSL),
                    (wv_d, wv_sb, KD, NSL),
                    (wo_d, wo_sb, NT, D),
                ):
                    wst = wsp.tile([P, wkd, wn], F32, tag="wst")
                    nc.sync.dma_start(
                        wst, w_d.rearrange("(t p) n -> p t n", p=P)
                    )
                    nc.vector.tensor_copy(w_sb, wst)

            # qT/o_cat are per-i-block tensors so attention / out-projection
            # dependencies stay block-granular (enables cross-phase overlap).
            qT_b = [
                pp.tile([P, NT, SBLK], F32R, name=f"qT{b}") for b in range(NB)
            ]
            kT = pp.tile([P, NT, S], F32R)
            ones_colf = pp.tile([1, DH], F32)
            nc.vector.memset(ones_colf, 1.0)
            ones_col = pp.tile([1, DH], F32R)
            nc.vector.tensor_copy(ones_col, ones_colf)
            v_sb = pp.tile([P, JT, NH, DH + 1], F32R)  # natural V + ones col
            vonesf = pp.tile([P, JT, NH, 1], F32)
            nc.vector.memset(vonesf, 1.0)
            nc.vector.tensor_copy(v_sb[:, :, :, DH : DH + 1], vonesf)
            o_b = [
                pp.tile([P, NT, SBLK], F32R, name=f"o{b}") for b in range(NB)
            ]

            # ---- Phase A: transpose inputs + projections ----
            with tc.tile_pool(name="pha", bufs=2) as pa, \
                 tc.tile_pool(name="psa", bufs=2, space="PSUM") as psa:
                plans = [
                    (xk_d, wk_sb, bk_sb, None, "qk"),
                    (xv_d, wv_sb, None, None, "v"),
                    (xq_d, wq_sb, bq_sb, qT_b, "q"),
                ]
                for x_d, w_sb, b_sb, dstT, kind in plans:
                    for blk in range(NB):
                        xn = pa.tile([P, SS, D], F32, tag="xn")
                        nc.sync.dma_start(
                            xn,
                            x_d[blk * SBLK : (blk + 1) * SBLK].rearrange(
                                "(ss p) d -> p ss d", p=P
                            ),
                        )
                        xT = pa.tile([P, KD, SBLK], F32R, tag="xT")
                        for ss in range(SS):
                            for kd in range(KD):
                                pst = psa.tile([P, P], F32, tag="pst", bufs=4)
                                nc.tensor.transpose(
                                    pst,
                                    xn[:, ss, kd * P : (kd + 1) * P],
                                    ident,
                                )
                                nc.vector.tensor_copy(
                                    xT[:, kd, ss * P : (ss + 1) * P], pst
                                )
                        if kind in ("qk", "q"):
                            for nt in range(NT):
                                psp = psa.tile([P, SBLK], F32, tag="psp")
                                for kd in range(KD):
                                    nc.tensor.matmul(
                                        psp,
                                        lhsT=w_sb[:, kd, nt * P : (nt + 1) * P],
                                        rhs=xT[:, kd, :],
                                        start=(kd == 0),
                                        stop=(kd == KD - 1),
                                    )
                                dst = (
                                    dstT[blk][:, nt, :]
                                    if kind == "q"
                                    else kT[:, nt, blk * SBLK : (blk + 1) * SBLK]
                                )
                                nc.vector.tensor_scalar_add(
                                    dst, psp, b_sb[:, nt : nt + 1]
                                )
                        else:
                            for ss in range(SS):
                                psv = psa.tile([P, NSL], F32, tag="psv")
                                for kd in range(KD):
                                    nc.tensor.matmul(
                                        psv,
                                        lhsT=xT[:, kd, ss * P : (ss + 1) * P],
                                        rhs=wv_sb[:, kd, :],
                                        start=(kd == 0),
                                        stop=(kd == KD - 1),
                                    )
                                st = blk * SS + ss
                                nc.vector.tensor_copy(
                                    v_sb[:, st, :, 0:DH],
                                    psv.rearrange("p (h d) -> p h d", d=DH),
                                )

            # ---- Phase B: attention per i-block, per head-pair; the output
            # projection for each finished i-block is fused in as dense PE
            # filler (keeps the HAM clock warm through the ACT-paced chunks).
            with tc.tile_pool(name="phb", bufs=2) as pb, \
                 tc.tile_pool(name="psb", bufs=1, space="PSUM") as psb:

                def dummy_mm():
                    # Dependency-free matmul on resident weights: absorbs PE
                    # idle slack so the HAM clock gate stays at full speed.
                    scr = psb.tile([P, SBLK], F32, tag="scr", bufs=1,
                                   name="scr")
                    nc.tensor.matmul(
                        scr, lhsT=wo_sb[:, 0, 0:P], rhs=wo_sb[:, 1, 0:SBLK],
                        start=True, stop=True,
                    )

                opq_state = {}

                def make_oproj_ops(ib):
                    # One emitter per (st, nb, t): the previous i-block's
                    # output projection, spread across attention chunks as
                    # real PE filler work.
                    ops = []
                    for st in range(ib * SS, (ib + 1) * SS):
                        for nb in range(D // SBLK):
                            for t in range(NT):
                                def emit(ib=ib, st=st, nb=nb, t=t):
                                    if t == 0:
                                        opq_state["pso"] = psb.tile(
                                            [P, SBLK], F32, tag="pso",
                                            bufs=1, name="pso",
                                        )
                                    pso = opq_state["pso"]
                                    ss_off = (st - ib * SS) * P
                                    nc.tensor.matmul(
                                        pso,
                                        lhsT=o_b[ib][:, t, ss_off : ss_off + P],
                                        rhs=wo_sb[:, t,
                                                  nb * SBLK : (nb + 1) * SBLK],
                                        start=(t == 0),
                                        stop=(t == NT - 1),
                                    )
                                    if t == NT - 1:
                                        ob = pb.tile([P, SBLK], F32,
                                                     tag="ob", bufs=3,
                                                     name="ob")
                                        nc.vector.tensor_copy(ob, pso)
                                        nc.sync.dma_start(
                                            out_d[st * P : (st + 1) * P,
                                                  nb * SBLK : (nb + 1) * SBLK],
                                            ob,
                                        )
                                ops.append(emit)
                    return ops

                oproj_queue = []
                for ib in range(NB):
                    for hp in range(NT):
                        ps_o = [
                            psb.tile([P, SBLK], F32, tag=f"ps_o{h01}",
                                     bufs=1, name=f"ps_o{h01}")
                            for h01 in range(2)
                        ]
                        for jc in range(JT // JC):
                            ps_s = [
                                psb.tile([P, JC, SBLK], F32, tag="ps_s",
                                         bufs=2, name=f"ps_s{h01}")
                                for h01 in range(2)
                            ]
                            for jj in range(JC):
                                jt = jc * JC + jj
                                for h01 in range(2):
                                    base = h01 * DH
                                    nc.tensor.matmul(
                                        ps_s[h01][:, jj, :],
                                        lhsT=kT[base : base + DH, hp,
                                                jt * P : (jt + 1) * P],
                                        rhs=qT_b[ib][base : base + DH, hp, :],
                                        start=True,
                                        stop=True,
                                        tile_position=(base, 0),
                                    )
                            for h01 in range(2):
                                h = hp * 2 + h01
                                p_sb = pb.tile([P, JC, SBLK], F32R,
                                               tag=f"p_sb{h01}", bufs=3)
                                nc.scalar.activation(
                                    p_sb, ps_s[h01],
                                    mybir.ActivationFunctionType.Exp,
                                    scale=float(1.0 / np.sqrt(DH)),
                                )
                                for jj in range(JC):
                                    jt = jc * JC + jj
                                    nc.tensor.matmul(
                                        ps_o[h01][0 : DH + 1, :],
                                        lhsT=v_sb[:, jt, h, :],
                                        rhs=p_sb[:, jj, :],
                                        start=(jt == 0),
                                        stop=(jt == JT - 1),
                                    )
                            if oproj_queue:
                                oproj_queue.pop(0)()
                            dummy_mm()
                        for h01 in range(2):
                            base = h01 * DH
                            recf = pb.tile([1, SBLK], F32, tag="recf", bufs=2)
                            nc.vector.reciprocal(recf, ps_o[h01][DH : DH + 1, :])
                            rec = pb.tile([1, SBLK], F32R, tag="rec", bufs=2)
                            nc.vector.tensor_copy(rec, recf)
                            ps_b = psb.tile([P, JC, SBLK], F32, tag="ps_s",
                                            bufs=2, name="ps_b")[0:DH, 0, :]
                            nc.tensor.matmul(
                                ps_b, lhsT=ones_col, rhs=rec,
                                start=True, stop=True,
                            )
                            bc = pb.tile([DH, SBLK], F32, tag="bc", bufs=2)
                            nc.vector.tensor_copy(bc, ps_b)
                            o_slice = o_b[ib][base : base + DH, hp, :]
                            nc.vector.tensor_mul(o_slice, ps_o[h01][0:DH, :], bc)
                            nc.vector.tensor_scalar_add(
                                o_slice, o_slice,
                                bv_sb[base : base + DH, hp : hp + 1],
                            )
                        dummy_mm()
                        dummy_mm()

                    # drain any leftover filler, then queue this block's
                    # output projection for the next block's chunks
                    for op in oproj_queue:
                        op()
                    oproj_queue = make_oproj_ops(ib)
                for op in oproj_queue:
                    op()

    nc.finalize()
    return nc


_NC_CACHE = {}


def _get_program(S, D, NH):
    key = (S, D, NH)
    if key not in _NC_CACHE:
        _NC_CACHE[key] = build_core_program(S, D, NH)
    return _NC_CACHE[key]


def kernel(q, k, v, Wq, bq, Wk, bk, Wv, bv, Wo, bo):
    q, k, v = (np.asarray(x, np.float32) for x in (q, k, v))
    Wq, Wk, Wv, Wo = (np.asarray(x, np.float32) for x in (Wq, Wk, Wv, Wo))
    bq, bk, bv, bo = (np.asarray(x, np.float32) for x in (bq, bk, bv, bo))
    B, S, D = q.shape
    GROUPS = N_CORES // B
    NSL = D // GROUPS

    nc = _get_program(S, D, NSL // DH)

    in_maps = []
    for c in range(N_CORES):
        b, hg = c // GROUPS, c % GROUPS
        sl = slice(hg * NSL, (hg + 1) * NSL)
        in_maps.append(
            {
                "xq": q[b],
                "xk": k[b],
                "xv": v[b],
                "wq": np.ascontiguousarray(Wq[:, sl]),
                "wk": np.ascontiguousarray(Wk[:, sl]),
                "wv": np.ascontiguousarray(Wv[:, sl]),
                "wo": np.ascontiguousarray(Wo[sl, :]),
                "bq": np.ascontiguousarray(bq[sl]),
                "bk": np.ascontiguousarray(bk[sl]),
                "bv": np.ascontiguousarray(bv[sl]),
            }
        )

    res = run_bass_kernel_spmd(nc, in_maps, list(range(N_CORES)))

    out = np.zeros((B, S, D), np.float32)
    for c in range(N_CORES):
        b = c // GROUPS
        out[b] += res.results[c]["out"]
    out += bo[None, None, :]
    return out


# revision 38
# speedup vs baseline: 1.0757x; 1.0757x over previous
"""Multi-head attention (B=2, S=2048, D=1024, H=16) on 8 Trainium2 cores.

Sharding: core c = (batch b, head-group hg) with b = c // 4, hg = c % 4.
Each core computes 4 heads of one batch element end-to-end:
  - Q^T/K^T projections in [dh, s] layout (scores computed transposed so the
    softmax denominator comes out of the PV matmul via a ones-column in V)
  - V projection in natural [s, dh] layout
  - exp on ScalarE with the 1/sqrt(dh) scale fused into the activation
  - partial output projection against the core's row-slice of Wo
Host sums the 4 partial projections per batch and adds bo.

Matmuls run as float32r (full-rate fp32 path on the PE for moving dim >= 256);
accumulation is always fp32 in PSUM. Walrus requires fp32r matmul operands to
be produced by an instruction that rounds to fp32r, so every matmul input tile
is allocated with dtype float32r and written by a DVE/ACT op (the rounding is
fused into copies we need anyway). Input transposes run in plain fp32.
"""

import numpy as np

import concourse.bacc as bacc
import concourse.mybir as mybir
import concourse.tile as tile
from concourse.bass_utils import run_bass_kernel_spmd
from concourse.masks import make_identity

F32 = mybir.dt.float32
F32R = mybir.dt.float32r

S_FULL, D_FULL, NH_PER_CORE, DH = 2048, 1024, 4, 64
N_CORES = 8
B_FULL, H_FULL = 2, 16


def build_core_program(S=S_FULL, D=D_FULL, NH=NH_PER_CORE):
    """One core's program: inputs xq/xk/xv [S,D], weight slices wq/wk/wv
    [D,NSL], wo [NSL,D], biases [NSL]; output out [S,D] (partial sum)."""
    NSL = NH * DH            # projection slice width for this core
    P = 128
    KD = D // P              # d-tiles (contraction tiles for projections)
    NT = NSL // P            # n-tiles = head-pairs
    ST = S // P              # s-tiles
    SBLK = 512 if S % 512 == 0 else S
    NB = S // SBLK           # s/i blocks
    JT = ST                  # j-tiles in attention
    JC = 2                   # j-tiles per score/exp chunk
    SS = SBLK // P           # s-subtiles per block

    nc = bacc.Bacc("TRN2", target_bir_lowering=False, debug=False)

    xq_d = nc.dram_tensor("xq", [S, D], F32, kind="ExternalInput")
    xk_d = nc.dram_tensor("xk", [S, D], F32, kind="ExternalInput")
    xv_d = nc.dram_tensor("xv", [S, D], F32, kind="ExternalInput")
    wq_d = nc.dram_tensor("wq", [D, NSL], F32, kind="ExternalInput")
    wk_d = nc.dram_tensor("wk", [D, NSL], F32, kind="ExternalInput")
    wv_d = nc.dram_tensor("wv", [D, NSL], F32, kind="ExternalInput")
    wo_d = nc.dram_tensor("wo", [NSL, D], F32, kind="ExternalInput")
    bq_d = nc.dram_tensor("bq", [NSL], F32, kind="ExternalInput")
    bk_d = nc.dram_tensor("bk", [NSL], F32, kind="ExternalInput")
    bv_d = nc.dram_tensor("bv", [NSL], F32, kind="ExternalInput")
    out_d = nc.dram_tensor("out", [S, D], F32, kind="ExternalOutput")

    with tile.TileContext(nc) as tc:
        with tc.tile_pool(name="persist", bufs=1) as pp:
            ident = pp.tile([P, P], F32)
            make_identity(nc, ident)


            # Weights: DMA fp32 staging -> rounded fp32r copies.
            wq_sb = pp.tile([P, KD, NSL], F32R)
            wk_sb = pp.tile([P, KD, NSL], F32R)
            wv_sb = pp.tile([P, KD, NSL], F32R)
            wo_sb = pp.tile([P, NT, D], F32R)
            bq_sb = pp.tile([P, NT], F32)
            nc.sync.dma_start(bq_sb, bq_d.rearrange("(t p) -> p t", p=P))
            bk_sb = pp.tile([P, NT], F32)
            nc.sync.dma_start(bk_sb, bk_d.rearrange("(t p) -> p t", p=P))
            bv_sb = pp.tile([P, NT], F32)
            nc.sync.dma_start(bv_sb, bv_d.rearrange("(t p) -> p t", p=P))

            with tc.tile_pool(name="wstage", bufs=2) as wsp:
                for w_d, w_sb, wkd, wn in (
                    (wq_d, wq_sb, KD, NSL),
                    (wk_d, wk_sb, KD, NSL),
                    (wv_d, wv_sb, KD, NSL),
                    (wo_d, wo_sb, NT, D),
                ):
                    wst = wsp.tile([P, wkd, wn], F32, tag="wst")
                    nc.sync.dma_start(
                        wst, w_d.rearrange("(t p) n -> p t n", p=P)
                    )
                    nc.vector.tensor_copy(w_sb, wst)

            # qT/o_cat are per-i-block tensors so attention / out-projection
            # dependencies stay block-granular (enables cross-phase overlap).
            qT_b = [
                pp.tile([P, NT, SBLK], F32R, name=f"qT{b}") for b in range(NB)
            ]
            kT = pp.tile([P, NT, S], F32R)
            ones_colf = pp.tile([1, DH], F32)
            nc.vector.memset(ones_colf, 1.0)
            ones_col = pp.tile([1, DH], F32R)
            nc.vector.tensor_copy(ones_col, ones_colf)
            v_sb = pp.tile([P, JT, NH, DH + 1], F32R)  # natural V + ones col
            vonesf = pp.tile([P, JT, NH, 1], F32)
            nc.vector.memset(vonesf, 1.0)
            nc.vector.tensor_copy(v_sb[:, :, :, DH : DH + 1], vonesf)
            o_b = [
                pp.tile([P, NT, SBLK], F32R, name=f"o{b}") for b in range(NB)
            ]

            # ---- Phase A: transpose inputs + projections ----
            with tc.tile_pool(name="pha", bufs=2) as pa, \
                 tc.tile_pool(name="psa", bufs=2, space="PSUM") as psa:
                plans = [
                    (xk_d, wk_sb, bk_sb, None, "qk"),
                    (xv_d, wv_sb, None, None, "v"),
                    (xq_d, wq_sb, bq_sb, qT_b, "q"),
                ]
                for x_d, w_sb, b_sb, dstT, kind in plans:
                    for blk in range(NB):
                        xn = pa.tile([P, SS, D], F32, tag="xn")
                        nc.sync.dma_start(
                            xn,
                            x_d[blk * SBLK : (blk + 1) * SBLK].rearrange(
                                "(ss p) d -> p ss d", p=P
                            ),
                        )
                        xT = pa.tile([P, KD, SBLK], F32R, tag="xT")
                        for ss in range(SS):
                            for kd in range(KD):
                                pst = psa.tile([P, P], F32, tag="pst", bufs=4)
                                nc.tensor.transpose(
                                    pst,
                                    xn[:, ss, kd * P : (kd + 1) * P],
                                    ident,
                                )
                                nc.vector.tensor_copy(
                                    xT[:, kd, ss * P : (ss + 1) * P], pst
                                )
                        if kind in ("qk", "q"):
                            for nt in range(NT):
                                psp = psa.tile([P, SBLK], F32, tag="psp")
                                for kd in range(KD):
                                    nc.tensor.matmul(
                                        psp,
                                        lhsT=w_sb[:, kd, nt * P : (nt + 1) * P],
                                        rhs=xT[:, kd, :],
                                        start=(kd == 0),
                                        stop=(kd == KD - 1),
                                    )
                                dst = (
                                    dstT[blk][:, nt, :]
                                    if kind == "q"
                                    else kT[:, nt, blk * SBLK : (blk + 1) * SBLK]
                                )
                                nc.vector.tensor_scalar_add(
                                    dst, psp, b_sb[:, nt : nt + 1]
                                )
                        else:
                            for ss in range(SS):
                                psv = psa.tile([P, NSL], F32, tag="psv")
                                for kd in range(KD):
                                    nc.tensor.matmul(
                                        psv,
                                        lhsT=xT[:, kd, ss * P : (ss + 1) * P],
                                        rhs=wv_sb[:, kd, :],
                                        start=(kd == 0),
                                        stop=(kd == KD - 1),
                                    )
                                st = blk * SS + ss
                                nc.vector.tensor_copy(
                                    v_sb[:, st, :, 0:DH],
                                    psv.rearrange("p (h d) -> p h d", d=DH),
                                )

            # ---- Phase B: attention per i-block, per head-pair; the output
            # projection for each finished i-block is fused in as dense PE
            # filler (keeps the HAM clock warm through the ACT-paced chunks).
            with tc.tile_pool(name="phb", bufs=2) as pb, \
                 tc.tile_pool(name="psb", bufs=1, space="PSUM") as psb:

                def make_oproj_ops(ib):
                    # One emitter per (st, nb): a dense 2-matmul group of the
                    # i-block's output projection. Emitted in bursts between
                    # attention sections as PE-warming filler.
                    ops = []
                    for st in range(ib * SS, (ib + 1) * SS):
                        for nb in range(D // SBLK):
                            def emit(ib=ib, st=st, nb=nb):
                                pso = psb.tile([P, SBLK], F32, tag="pso",
                                               bufs=2, name="pso")
                                ss_off = (st - ib * SS) * P
                                for t in range(NT):
                                    nc.tensor.matmul(
                                        pso,
                                        lhsT=o_b[ib][:, t, ss_off : ss_off + P],
                                        rhs=wo_sb[:, t,
                                                  nb * SBLK : (nb + 1) * SBLK],
                                        start=(t == 0),
                                        stop=(t == NT - 1),
                                    )
                                ob = pb.tile([P, SBLK], F32, tag="ob",
                                             bufs=3, name="ob")
                                nc.vector.tensor_copy(ob, pso)
                                nc.sync.dma_start(
                                    out_d[st * P : (st + 1) * P,
                                          nb * SBLK : (nb + 1) * SBLK],
                                    ob,
                                )
                            ops.append(emit)
                    return ops

                oproj_queue = []
                for ib in range(NB):
                    for hp in range(NT):
                        ps_o = [
                            psb.tile([P, SBLK], F32, tag=f"ps_o{h01}",
                                     bufs=1, name=f"ps_o{h01}")
                            for h01 in range(2)
                        ]
                        for jc in range(JT // JC):
                            ps_s = [
                                psb.tile([P, JC, SBLK], F32, tag="ps_s",
                                         bufs=2, name=f"ps_s{h01}")
                                for h01 in range(2)
                            ]
                            for jj in range(JC):
                                jt = jc * JC + jj
                                for h01 in range(2):
                                    base = h01 * DH
                                    nc.tensor.matmul(
                                        ps_s[h01][:, jj, :],
                                        lhsT=kT[base : base + DH, hp,
                                                jt * P : (jt + 1) * P],
                                        rhs=qT_b[ib][base : base + DH, hp, :],
                                        start=True,
                                        stop=True,
                                        tile_position=(base, 0),
                                    )
                            for h01 in range(2):
                                h = hp * 2 + h01
                                p_sb = pb.tile([P, JC, SBLK], F32R,
                                               tag=f"p_sb{h01}", bufs=3)
                                nc.scalar.activation(
                                    p_sb, ps_s[h01],
                                    mybir.ActivationFunctionType.Exp,
                                    scale=float(1.0 / np.sqrt(DH)),
                                )
                                for jj in range(JC):
                                    jt = jc * JC + jj
                                    nc.tensor.matmul(
                                        ps_o[h01][0 : DH + 1, :],
                                        lhsT=v_sb[:, jt, h, :],
                                        rhs=p_sb[:, jj, :],
                                        start=(jt == 0),
                                        stop=(jt == JT - 1),
                                    )
                        for h01 in range(2):
                            base = h01 * DH
                            recf = pb.tile([1, SBLK], F32, tag="recf", bufs=2)
                            nc.vector.reciprocal(recf, ps_o[h01][DH : DH + 1, :])
                            rec = pb.tile([1, SBLK], F32R, tag="rec", bufs=2)
                            nc.vector.tensor_copy(rec, recf)
                            ps_b = psb.tile([P, JC, SBLK], F32, tag="ps_s",
                                            bufs=2, name="ps_b")[0:DH, 0, :]
                            nc.tensor.matmul(
                                ps_b, lhsT=ones_col, rhs=rec,
                                start=True, stop=True,
                            )
                            bc = pb.tile([DH, SBLK], F32, tag="bc", bufs=2)
                            nc.vector.tensor_copy(bc, ps_b)
                            o_slice = o_b[ib][base : base + DH, hp, :]
                            nc.vector.tensor_mul(o_slice, ps_o[h01][0:DH, :], bc)
                            nc.vector.tensor_scalar_add(
                                o_slice, o_slice,
                                bv_sb[base : base + DH, hp : hp + 1],
                            )

                        # dense out-proj burst (previous i-block) over the
                        # normalize boundary, keeping the PE clock warm
                        half = max(1, len(oproj_queue) // 2)
                        for op in oproj_queue[:half]:
                            op()
                        oproj_queue = oproj_queue[half:]

                    for op in oproj_queue:
                        op()
                    oproj_queue = make_oproj_ops(ib)
                for op in oproj_queue:
                    op()

    nc.finalize()
    return nc


_NC_CACHE = {}


def _get_program(S, D, NH):
    key = (S, D, NH)
    if key not in _NC_CACHE:
        _NC_CACHE[key] = build_core_program(S, D, NH)
    return _NC_CACHE[key]


def kernel(q, k, v, Wq, bq, Wk, bk, Wv, bv, Wo, bo):
    q, k, v = (np.asarray(x, np.float32) for x in (q, k, v))
    Wq, Wk, Wv, Wo = (np.asarray(x, np.float32) for x in (Wq, Wk, Wv, Wo))
    bq, bk, bv, bo = (np.asarray(x, np.float32) for x in (bq, bk, bv, bo))
    B, S, D = q.shape
    GROUPS = N_CORES // B
    NSL = D // GROUPS

    nc = _get_program(S, D, NSL // DH)

    in_maps = []
    for c in range(N_CORES):
        b, hg = c // GROUPS, c % GROUPS
        sl = slice(hg * NSL, (hg + 1) * NSL)
        in_maps.append(
            {
                "xq": q[b],
                "xk": k[b],
                "xv": v[b],
                "wq": np.ascontiguousarray(Wq[:, sl]),
                "wk": np.ascontiguousarray(Wk[:, sl]),
                "wv": np.ascontiguousarray(Wv[:, sl]),
                "wo": np.ascontiguousarray(Wo[sl, :]),
                "bq": np.ascontiguousarray(bq[sl]),
                "bk": np.ascontiguousarray(bk[sl]),
                "bv": np.ascontiguousarray(bv[sl]),
            }
        )

    res = run_bass_kernel_spmd(nc, in_maps, list(range(N_CORES)))

    out = np.zeros((B, S, D), np.float32)
    for c in range(N_CORES):
        b = c // GROUPS
        out[b] += res.results[c]["out"]
    out += bo[None, None, :]
    return out


# revision 42
# speedup vs baseline: 1.3343x; 1.2404x over previous
"""Multi-head attention (B=2, S=2048, D=1024, H=16) on 8 Trainium2 cores.

Sharding: core c = (batch b, head-group hg) with b = c // 4, hg = c % 4.
Each core computes 4 heads of one batch element end-to-end:
  - Q^T/K^T projections in [dh, s] layout (scores computed transposed so the
    softmax denominator comes out of the PV matmul via a ones-column in V)
  - V projection in natural [s, dh] layout
  - exp on ScalarE with the 1/sqrt(dh) scale fused into the activation
  - partial output projection against the core's row-slice of Wo
Host sums the 4 partial projections per batch and adds bo.

Matmuls run as float32r (full-rate fp32 path on the PE for moving dim >= 256);
accumulation is always fp32 in PSUM. Walrus requires fp32r matmul operands to
be produced by an instruction that rounds to fp32r, so every matmul input tile
is allocated with dtype float32r and written by a DVE/ACT op (the rounding is
fused into copies we need anyway). Input transposes run in plain fp32.
"""

import numpy as np

import concourse.bacc as bacc
import concourse.mybir as mybir
import concourse.tile as tile
from concourse.bass_utils import run_bass_kernel_spmd
from concourse.masks import make_identity

F32 = mybir.dt.float32
F32R = mybir.dt.float32r

S_FULL, D_FULL, NH_PER_CORE, DH = 2048, 1024, 4, 64
N_CORES = 8
B_FULL, H_FULL = 2, 16


def build_core_program(S=S_FULL, D=D_FULL, NH=NH_PER_CORE):
    """One core's program: inputs xq/xk/xv [S,D], weight slices wq/wk/wv
    [D,NSL], wo [NSL,D], biases [NSL]; output out [S,D] (partial sum)."""
    NSL = NH * DH            # projection slice width for this core
    P = 128
    KD = D // P              # d-tiles (contraction tiles for projections)
    NT = NSL // P            # n-tiles = head-pairs
    ST = S // P              # s-tiles
    SBLK = 512 if S % 512 == 0 else S
    NB = S // SBLK           # s/i blocks
    JT = ST                  # j-tiles in attention
    JC = 2                   # j-tiles per score/exp chunk
    SS = SBLK // P           # s-subtiles per block

    nc = bacc.Bacc("TRN2", target_bir_lowering=False, debug=False)

    xq_d = nc.dram_tensor("xq", [S, D], F32, kind="ExternalInput")
    xk_d = nc.dram_tensor("xk", [S, D], F32, kind="ExternalInput")
    xv_d = nc.dram_tensor("xv", [S, D], F32, kind="ExternalInput")
    wq_d = nc.dram_tensor("wq", [D, NSL], F32, kind="ExternalInput")
    wk_d = nc.dram_tensor("wk", [D, NSL], F32, kind="ExternalInput")
    wv_d = nc.dram_tensor("wv", [D, NSL], F32, kind="ExternalInput")
    wo_d = nc.dram_tensor("wo", [NSL, D], F32, kind="ExternalInput")
    bq_d = nc.dram_tensor("bq", [NSL], F32, kind="ExternalInput")
    bk_d = nc.dram_tensor("bk", [NSL], F32, kind="ExternalInput")
    bv_d = nc.dram_tensor("bv", [NSL], F32, kind="ExternalInput")
    out_d = nc.dram_tensor("out", [S, D], F32, kind="ExternalOutput")

    with tile.TileContext(nc) as tc:
        with tc.tile_pool(name="persist", bufs=1) as pp:
            ident = pp.tile([P, P], F32)
            make_identity(nc, ident)


            # Weights: DMA fp32 staging -> rounded fp32r copies.
            wq_sb = pp.tile([P, KD, NSL], F32R)
            wk_sb = pp.tile([P, KD, NSL], F32R)
            wv_sb = pp.tile([P, KD, NSL], F32R)
            wo_sb = pp.tile([P, NT, D], F32R)
            bq_sb = pp.tile([P, NT], F32)
            nc.sync.dma_start(bq_sb, bq_d.rearrange("(t p) -> p t", p=P))
            bk_sb = pp.tile([P, NT], F32)
            nc.sync.dma_start(bk_sb, bk_d.rearrange("(t p) -> p t", p=P))
            bv_sb = pp.tile([P, NT], F32)
            nc.sync.dma_start(bv_sb, bv_d.rearrange("(t p) -> p t", p=P))

            with tc.tile_pool(name="wstage", bufs=2) as wsp:
                for w_d, w_sb, wkd, wn in (
                    (wq_d, wq_sb, KD, NSL),
                    (wk_d, wk_sb, KD, NSL),
                    (wv_d, wv_sb, KD, NSL),
                    (wo_d, wo_sb, NT, D),
                ):
                    wst = wsp.tile([P, wkd, wn], F32, tag="wst")
                    nc.sync.dma_start(
                        wst, w_d.rearrange("(t p) n -> p t n", p=P)
                    )
                    nc.vector.tensor_copy(w_sb, wst)

            # qT/o_cat are per-i-block tensors so attention / out-projection
            # dependencies stay block-granular (enables cross-phase overlap).
            qT_b = [
                pp.tile([P, NT, SBLK], F32R, name=f"qT{b}") for b in range(NB)
            ]
            kT = pp.tile([P, NT, S], F32R)
            ones_colf = pp.tile([1, DH], F32)
            nc.vector.memset(ones_colf, 1.0)
            ones_col = pp.tile([1, DH], F32R)
            nc.vector.tensor_copy(ones_col, ones_colf)
            v_sb = pp.tile([P, JT, NH, DH + 1], F32R)  # natural V + ones col
            vonesf = pp.tile([P, JT, NH, 1], F32)
            nc.vector.memset(vonesf, 1.0)
            nc.vector.tensor_copy(v_sb[:, :, :, DH : DH + 1], vonesf)
            o_b = [
                pp.tile([P, NT, SBLK], F32R, name=f"o{b}") for b in range(NB)
            ]

            # ---- Phase A: transpose inputs + projections ----
            with tc.tile_pool(name="pha", bufs=2) as pa, \
                 tc.tile_pool(name="psa", bufs=2, space="PSUM") as psa:
                plans = [
                    (xk_d, wk_sb, bk_sb, None, "qk"),
                    (xv_d, wv_sb, None, None, "v"),
                    (xq_d, wq_sb, bq_sb, qT_b, "q"),
                ]
                for x_d, w_sb, b_sb, dstT, kind in plans:
                    for blk in range(NB):
                        xn = pa.tile([P, SS, D], F32, tag="xn")
                        nc.sync.dma_start(
                            xn,
                            x_d[blk * SBLK : (blk + 1) * SBLK].rearrange(
                                "(ss p) d -> p ss d", p=P
                            ),
                        )
                        xT = pa.tile([P, KD, SBLK], F32R, tag="xT")
                        for ss in range(SS):
                            for kd in range(KD):
                                pst = psa.tile([P, P], F32, tag="pst", bufs=4)
                                nc.tensor.transpose(
                                    pst,
                                    xn[:, ss, kd * P : (kd + 1) * P],
                                    ident,
                                )
                                nc.vector.tensor_copy(
                                    xT[:, kd, ss * P : (ss + 1) * P], pst
                                )
                        if kind in ("qk", "q"):
                            for nt in range(NT):
                                psp = psa.tile([P, SBLK], F32, tag="psp")
                                for kd in range(KD):
                                    nc.tensor.matmul(
                                        psp,
                                        lhsT=w_sb[:, kd, nt * P : (nt + 1) * P],
                                        rhs=xT[:, kd, :],
                                        start=(kd == 0),
                                        stop=(kd == KD - 1),
                                    )
                                dst = (
                                    dstT[blk][:, nt, :]
                                    if kind == "q"
                                    else kT[:, nt, blk * SBLK : (blk + 1) * SBLK]
                                )
                                nc.vector.tensor_scalar_add(
                                    dst, psp, b_sb[:, nt : nt + 1]
                                )
                        else:
                            for ss in range(SS):
                                psv = psa.tile([P, NSL], F32, tag="psv")
                                for kd in range(KD):
                                    nc.tensor.matmul(
                                        psv,
                                        lhsT=xT[:, kd, ss * P : (ss + 1) * P],
                                        rhs=wv_sb[:, kd, :],
                                        start=(kd == 0),
                                        stop=(kd == KD - 1),
                                    )
                                st = blk * SS + ss
                                nc.vector.tensor_copy(
                                    v_sb[:, st, :, 0:DH],
                                    psv.rearrange("p (h d) -> p h d", d=DH),
                                )

            # ---- Phase B: attention per i-block, per head-pair; the output
            # projection for each finished i-block is fused in as dense PE
            # filler (keeps the HAM clock warm through the ACT-paced chunks).
            with tc.tile_pool(name="phb", bufs=2) as pb, \
                 tc.tile_pool(name="psb", bufs=1, space="PSUM") as psb:
                for ib in range(NB):
                    for hp in range(NT):
                        ps_o = [
                            psb.tile([P, SBLK], F32, tag=f"ps_o{h01}",
                                     bufs=1, name=f"ps_o{h01}")
                            for h01 in range(2)
                        ]

                        def emit_exp_pv(jc, ps_s):
                            for h01 in range(2):
                                h = hp * 2 + h01
                                p_sb = pb.tile([P, JC, SBLK], F32R,
                                               tag=f"p_sb{h01}", bufs=3,
                                               name="p_sb")
                                nc.scalar.activation(
                                    p_sb, ps_s[h01],
                                    mybir.ActivationFunctionType.Exp,
                                    scale=float(1.0 / np.sqrt(DH)),
                                )
                                for jj in range(JC):
                                    jt = jc * JC + jj
                                    nc.tensor.matmul(
                                        ps_o[h01][0 : DH + 1, :],
                                        lhsT=v_sb[:, jt, h, :],
                                        rhs=p_sb[:, jj, :],
                                        start=(jt == 0),
                                        stop=(jt == JT - 1),
                                    )

                        prev = None
                        for jc in range(JT // JC):
                            ps_s = [
                                psb.tile([P, JC, SBLK], F32, tag="ps_s",
                                         bufs=3, name=f"ps_s{h01}")
                                for h01 in range(2)
                            ]
                            for jj in range(JC):
                                jt = jc * JC + jj
                                for h01 in range(2):
                                    base = h01 * DH
                                    nc.tensor.matmul(
                                        ps_s[h01][:, jj, :],
                                        lhsT=kT[base : base + DH, hp,
                                                jt * P : (jt + 1) * P],
                                        rhs=qT_b[ib][base : base + DH, hp, :],
                                        start=True,
                                        stop=True,
                                        tile_position=(base, 0),
                                    )
                            if prev is not None:
                                emit_exp_pv(*prev)
                            prev = (jc, ps_s)
                        emit_exp_pv(*prev)
                        for h01 in range(2):
                            base = h01 * DH
                            recf = pb.tile([1, SBLK], F32, tag="recf", bufs=2)
                            nc.vector.reciprocal(recf, ps_o[h01][DH : DH + 1, :])
                            rec = pb.tile([1, SBLK], F32R, tag="rec", bufs=2)
                            nc.vector.tensor_copy(rec, recf)
                            ps_b = psb.tile([P, JC, SBLK], F32, tag="ps_s",
                                            bufs=3, name="ps_b")[0:DH, 0, :]
                            nc.tensor.matmul(
                                ps_b, lhsT=ones_col, rhs=rec,
                                start=True, stop=True,
                            )
                            bc = pb.tile([DH, SBLK], F32, tag="bc", bufs=2)
                            nc.vector.tensor_copy(bc, ps_b)
                            o_slice = o_b[ib][base : base + DH, hp, :]
                            nc.vector.tensor_mul(o_slice, ps_o[h01][0:DH, :], bc)
                            nc.vector.tensor_scalar_add(
                                o_slice, o_slice,
                                bv_sb[base : base + DH, hp : hp + 1],
                            )

                    # output projection for this finished i-block
                    for st in range(ib * SS, (ib + 1) * SS):
                        for nb in range(D // SBLK):
                            pso = psb.tile([P, SBLK], F32, tag="ps_o0",
                                           bufs=1, name="pso")
                            for t in range(NT):
                                ss_off = (st - ib * SS) * P
                                nc.tensor.matmul(
                                    pso,
                                    lhsT=o_b[ib][:, t, ss_off : ss_off + P],
                                    rhs=wo_sb[:, t, nb * SBLK : (nb + 1) * SBLK],
                                    start=(t == 0),
                                    stop=(t == NT - 1),
                                )
                            ob = pb.tile([P, SBLK], F32, tag="ob", bufs=3)
                            nc.vector.tensor_copy(ob, pso)
                            nc.sync.dma_start(
                                out_d[st * P : (st + 1) * P,
                                      nb * SBLK : (nb + 1) * SBLK],
                                ob,
                            )

    nc.finalize()
    return nc


_NC_CACHE = {}


def _get_program(S, D, NH):
    key = (S, D, NH)
    if key not in _NC_CACHE:
        _NC_CACHE[key] = build_core_program(S, D, NH)
    return _NC_CACHE[key]


def kernel(q, k, v, Wq, bq, Wk, bk, Wv, bv, Wo, bo):
    q, k, v = (np.asarray(x, np.float32) for x in (q, k, v))
    Wq, Wk, Wv, Wo = (np.asarray(x, np.float32) for x in (Wq, Wk, Wv, Wo))
    bq, bk, bv, bo = (np.asarray(x, np.float32) for x in (bq, bk, bv, bo))
    B, S, D = q.shape
    GROUPS = N_CORES // B
    NSL = D // GROUPS

    nc = _get_program(S, D, NSL // DH)

    in_maps = []
    for c in range(N_CORES):
        b, hg = c // GROUPS, c % GROUPS
        sl = slice(hg * NSL, (hg + 1) * NSL)
        in_maps.append(
            {
                "xq": q[b],
                "xk": k[b],
                "xv": v[b],
                "wq": np.ascontiguousarray(Wq[:, sl]),
                "wk": np.ascontiguousarray(Wk[:, sl]),
                "wv": np.ascontiguousarray(Wv[:, sl]),
                "wo": np.ascontiguousarray(Wo[sl, :]),
                "bq": np.ascontiguousarray(bq[sl]),
                "bk": np.ascontiguousarray(bk[sl]),
                "bv": np.ascontiguousarray(bv[sl]),
            }
        )

    res = run_bass_kernel_spmd(nc, in_maps, list(range(N_CORES)))

    out = np.zeros((B, S, D), np.float32)
    for c in range(N_CORES):
        b = c // GROUPS
        out[b] += res.results[c]["out"]
    out += bo[None, None, :]
    return out
